# revision 42
# baseline (speedup 1.0000x reference)
"""AttnBlock (GroupNorm + single-head 4096-token attention + residual) on 8
Trainium2 NeuronCores.

Sharding: core i handles batch b = i // 2 and query-half h = i % 2.  The host
permutes each batch's 4096 spatial tokens so the core's 2048 query tokens come
first; GroupNorm stats and the softmax sum are permutation-invariant, so K/V
use all 4096 tokens in permuted order and results are exact.

Device data flow per core.  Every large matmul runs in fp8e4m3 with
perf_mode=DoubleRow (contraction 256 per matmul, ~1.8x the bf16 rate at
free-dim 512).  Weights ship pre-scaled x16 on the host so their
~N(0, 1/512) entries sit in fp8's normal range; epilogues multiply by 1/16.
The softmax scale is folded into the exp (ACT computes exp(S/sqrt(C) - 3));
the -3 shift keeps the fp8 probabilities below TRN-e4m3's 240-overflow and
cancels in the normalization.  All loads are host-packed so each dma_start
is one descriptor per partition row (descriptor generation runs on the
issuing engine and would otherwise dominate the prologue).

  x[512,4096] (host-cast fp8; fp32 kept only for the residual) -> GroupNorm
  (per-channel sum on DVE + Square-accumulate on ACT with the h8 slots as
  scratch, exact fp32 indicator matmuls for the 16-channel group reduce,
  normalize split DVE/GPSIMD) -> h fp8 in DoubleRow pair layout [128, 2, N]
  -> Q^T (channel pairs), T = Wk^T Q (so S = K^T Q = h^T T: query-sized
  instead of key-sized, per-core local, and the k bias cancels in softmax),
  V (token pairs, h as the stationary operand) -> S^T = h^T T blocks in fp32
  PSUM -> exp(S*scale - 3) on ACT -> fp8 probability pairs -> softmax
  denominators via all-ones fp8 matmul on PE (replicated across partitions)
  -> A.V over 16 DoubleRow key pairs -> raw-po fp8 cast (1/denominator is
  per-query so it commutes past the output projection; the po PSUM banks
  free without waiting on the reciprocal) -> fp8 DoubleRow out-projection ->
  x invbc (computed as exp(-ln(sum)) on ACT) + residual (prefetched
  mid-S-loop) -> store.  Each chunk's epilogue is deferred into the next
  chunk's S-loop so its chain hides under matmuls; the last chunk's
  denominator matmul is hoisted first and its out-proj borrows the S pool's
  two PSUM banks.
"""

import contextlib

import ml_dtypes
import numpy as np

import concourse.bass as bass
import concourse.tile as tile
from concourse import mybir
from concourse.bass_utils import run_bass_kernel_spmd
from concourse.vector_clock import ScopedClock

F32 = mybir.dt.float32
BF16 = mybir.dt.bfloat16
FP8 = mybir.dt.float8e4
AF = mybir.ActivationFunctionType
DR = mybir.MatmulPerfMode.DoubleRow

B, C, H, W = 4, 512, 64, 64
N = H * W          # 4096 tokens
NQ = N // 2        # 2048 queries per core
P = 128
CT = C // P        # 4 channel tiles
CP = CT // 2       # 2 channel pair-tiles (DoubleRow)
NKT = N // P       # 32 key tiles
NKP = NKT // 2     # 16 key pair-tiles
QC = NQ // 512     # 4 query chunks of 512
GROUPS_PER_TILE = 8
GSIZE = 16         # channels per group
EPS = 1e-5
SCALE = float(C) ** -0.5
ESHIFT = -3.0      # exp(logit + ESHIFT): keeps fp8 probs < e4m3 max (240)
WSCALE = 16.0      # host multiplies q/k/v/o weights by this before fp8 cast
OSC = 32.0         # o = po * (OSC/psum) cast to fp8; final store divides it out
NSPAT = float(GSIZE * N)  # elements per group for GN stats


def _install_drain_split():
    """Walrus CTRL encoding fits one sync-wait per Drain; split the Tile
    kernel-tail drain's waits across several drains."""
    if getattr(tile.TileContext, "_drain_split_installed", False):
        return

    def _drain_and_barrier(self, tick_clock, wait_clock):
        drain_inst = self.nc.sync.drain()
        wait_clock.add_sem_waits(
            drain_inst.ins, ScopedClock({None: tick_clock.global_clock})
        )
        si = drain_inst.ins.sync_info
        if si is not None and len(si.on_wait) > 1:
            waits = list(si.on_wait)
            drain_inst.ins.sync_info = mybir.SyncInfo(
                on_wait=waits[:1], on_update=list(si.on_update)
            )
            for w in waits[1:]:
                extra = self.nc.sync.drain()
                extra.ins.sync_info = mybir.SyncInfo(on_wait=[w], on_update=[])

        self.nc.all_engine_barrier()
        assert self.sems is not None
        popped = self.nc._tile_sem_poison_stack.pop()
        assert popped is self._sem_poison
        self.nc.clear_and_free_semaphores(list(self.sems.allocated().values()))
        self.nc.all_engine_barrier()

    tile.TileContext._drain_and_barrier = _drain_and_barrier
    tile.TileContext._drain_split_installed = True


def _build_nc() -> bass.Bass:
    _install_drain_split()
    nc = bass.Bass()

    # All loads are host-packed so every dma_start is one descriptor per
    # partition row (descriptor generation runs on the issuing engine at
    # ~6 ns/descriptor and was the real prologue bottleneck).
    x_d = nc.declare_dram_parameter("x", [C, N], FP8, isOutput=False)
    xr_d = nc.declare_dram_parameter("xrp", [P, 16, 512], F32, isOutput=False)
    qw8_d = nc.declare_dram_parameter("qw8", [P, CT, C], FP8, isOutput=False)
    kw8_d = nc.declare_dram_parameter("kw8", [P, CT, C], FP8, isOutput=False)
    vw8_d = nc.declare_dram_parameter("vw8", [P, CT, C], FP8, isOutput=False)
    owT_d = nc.declare_dram_parameter("owT", [P, CT, C], FP8, isOutput=False)
    vecs_d = nc.declare_dram_parameter("vecs", [P, 12], F32, isOutput=False)
    ind_d = nc.declare_dram_parameter("ind", [P, GROUPS_PER_TILE], F32, isOutput=False)
    indT_d = nc.declare_dram_parameter("indT", [P, P], F32, isOutput=False)
    out_d = nc.declare_dram_parameter("out", [C, NQ], F32, isOutput=True)

    with tile.TileContext(nc) as tc, contextlib.ExitStack() as ctx:
        const = ctx.enter_context(tc.tile_pool(name="const", bufs=1))
        wpool = ctx.enter_context(tc.tile_pool(name="w", bufs=1))
        statp = ctx.enter_context(tc.tile_pool(name="stat", bufs=1))
        kvq = ctx.enter_context(tc.tile_pool(name="kvq", bufs=1))

        ps_s = ctx.enter_context(tc.tile_pool(name="ps_s", bufs=2, space="PSUM"))
        ps_o = ctx.enter_context(tc.tile_pool(name="ps_o", bufs=4, space="PSUM"))
        ps_stat = ctx.enter_context(tc.tile_pool(name="ps_stat", bufs=1, space="PSUM"))
        ps_out = ctx.enter_context(tc.tile_pool(name="ps_out", bufs=1, space="PSUM"))

        # ---- constants / parameter vectors (one packed DMA) ---------------
        vecs_sb = const.tile([P, 12], F32, tag="vecs")
        nc.sync.dma_start(out=vecs_sb[:], in_=vecs_d[:])
        gnw_sb = vecs_sb[:, 0:CT]
        gnb_sb = vecs_sb[:, CT : 2 * CT]
        qb_sb = vecs_sb[:, 2 * CT : 3 * CT]

        eps_sb = const.tile([P, 1], F32, tag="eps")
        nc.vector.memset(eps_sb, EPS)
        esh_sb = const.tile([P, 1], F32, tag="esh")
        nc.vector.memset(esh_sb, ESHIFT)
        lnc_sb = const.tile([P, 1], F32, tag="lnc")
        nc.vector.memset(lnc_sb, float(np.log(OSC)))
        ones8 = const.tile([P, 2, P], FP8, tag="ones8")
        nc.vector.memset(ones8, 1.0)

        # group indicator [128 ch, 8 groups] and padded transpose [128, 128]
        ind = const.tile([P, GROUPS_PER_TILE], F32, tag="ind")
        nc.scalar.dma_start(out=ind[:], in_=ind_d[:])
        indT = const.tile([P, P], F32, tag="indT")
        nc.scalar.dma_start(out=indT[:], in_=indT_d[:])

        # ---- fp8 weights, host-packed [P, 4, C]: [p, m, c] = w[m*128+p, c].
        # DoubleRow pair j is the slice [:, 2j:2j+2, :].
        def load_w8(dram, dtype=FP8):
            t = wpool.tile([P, CT, C], dtype, tag=f"w8_{dram.name}")
            nc.sync.dma_start(out=t[:], in_=dram[:])
            return [t[:, 2 * j : 2 * j + 2, :] for j in range(CP)]

        # ---- load x (resident, tiles serialized so stats chase the DMA) ----
        xh_ctx = contextlib.ExitStack()
        xpool = xh_ctx.enter_context(tc.tile_pool(name="xp", bufs=1))
        QT8 = [kvq.tile([P, 2, NQ], FP8, tag=f"QT{j}", name=f"QT{j}") for j in range(CP)]
        T8 = [kvq.tile([P, 2, NQ], FP8, tag=f"TT{j}", name=f"TT{j}") for j in range(CP)]
        V8 = [kvq.tile([P, 2, C], FP8, tag=f"VT{m}", name=f"VT{m}") for m in range(NKP)]

        # h8 persists through attention: it is the stationary operand of the
        # S matmuls (S = h^T (Wk^T Q))
        h8 = [
            kvq.tile([P, 2, N], FP8, tag=f"h{j}", name=f"h{j}") for j in range(CP)
        ]
        # One dma_start per x tile (row-contiguous: 128 descriptors each);
        # stats consume tiles in arrival order.
        # each channel tile as two independent half tiles on the two rings:
        # separate tiles guarantee the half's stats can start as soon as its
        # own DMA lands, whatever the dependency tracker's granularity
        HN = N // 2
        xt = [
            (
                xpool.tile([P, HN], FP8, tag=f"x{ct}a", name=f"x{ct}a"),
                xpool.tile([P, HN], FP8, tag=f"x{ct}b", name=f"x{ct}b"),
            )
            for ct in range(CT)
        ]
        for ct in range(CT):
            nc.sync.dma_start(
                out=xt[ct][0][:], in_=x_d[ct * P : (ct + 1) * P, 0:HN]
            )
            nc.scalar.dma_start(
                out=xt[ct][1][:], in_=x_d[ct * P : (ct + 1) * P, HN:]
            )
        # weights stream right behind x
        qw8 = load_w8(qw8_d)
        kw8 = load_w8(kw8_d)
        vw8 = load_w8(vw8_d)

        for ct in range(CT):
            ta, tb = xt[ct]
            # per-channel (sum, sumsq) per half tile as its DMA lands; the
            # h8 slot doubles as the squares scratch (real contents later)
            hslot = h8[ct // 2][:, ct % 2, :]
            st = statp.tile([P, 2], F32, tag=f"st{ct}")
            sth = statp.tile([P, 4], F32, tag=f"sth{ct}")
            nc.vector.reduce_sum(
                out=sth[:, 0:1], in_=ta[:], axis=mybir.AxisListType.X
            )
            nc.scalar.activation(
                out=hslot[:, 0:HN], in_=ta[:], func=AF.Square,
                accum_out=sth[:, 2:3],
            )
            nc.vector.reduce_sum(
                out=sth[:, 1:2], in_=tb[:], axis=mybir.AxisListType.X
            )
            nc.scalar.activation(
                out=hslot[:, HN:], in_=tb[:], func=AF.Square,
                accum_out=sth[:, 3:4],
            )
            nc.vector.tensor_add(st[:, 0:1], sth[:, 0:1], sth[:, 1:2])
            nc.vector.tensor_add(st[:, 1:2], sth[:, 2:3], sth[:, 3:4])

            # group reduce for this tile via exact fp32 matmuls
            psg = ps_stat.tile([GROUPS_PER_TILE, 2], F32, tag="stat", name=f"psg{ct}")
            nc.tensor.matmul(psg, ind, st, start=True, stop=True)
            gs = statp.tile([P, 2], F32, tag=f"gs{ct}")
            nc.vector.memset(gs, 0.0)
            nc.scalar.copy(out=gs[:GROUPS_PER_TILE, :], in_=psg[:])
            psc = ps_s.tile([P, 2], F32, tag="s", name=f"psc{ct}")
            nc.tensor.matmul(psc, indT, gs, start=True, stop=True)
            sm = statp.tile([P, 2], F32, tag=f"sm{ct}")
            nc.scalar.mul(out=sm[:], in_=psc, mul=1.0 / NSPAT)
            # mean^2 - E[x^2] in one fused op; the Sqrt's scale=-1 restores
            # the sign: rstd_in = sqrt(var + eps)
            negv = statp.tile([P, 1], F32, tag=f"nv{ct}")
            nc.vector.scalar_tensor_tensor(
                out=negv, in0=sm[:, 0:1], scalar=sm[:, 0:1], in1=sm[:, 1:2],
                op0=mybir.AluOpType.mult, op1=mybir.AluOpType.subtract,
            )
            rstd = statp.tile([P, 1], F32, tag=f"var{ct}")
            nc.scalar.activation(
                out=rstd, in_=negv, func=AF.Sqrt, bias=eps_sb[:, 0:1],
                scale=-1.0,
            )
            nc.vector.reciprocal(rstd, rstd)
            scl = statp.tile([P, 1], F32, tag=f"scl{ct}")
            nc.vector.tensor_mul(scl, rstd, vecs_sb[:, ct : ct + 1])
            # -(gn bias term): mean*scl - gnb; the normalize subtracts it
            nbs = statp.tile([P, 1], F32, tag=f"nb{ct}")
            nc.vector.scalar_tensor_tensor(
                out=nbs, in0=sm[:, 0:1], scalar=scl,
                in1=vecs_sb[:, CT + ct : CT + ct + 1],
                op0=mybir.AluOpType.mult, op1=mybir.AluOpType.subtract,
            )

            # normalize to h ((x*scl) - negbias), mostly on the
            # otherwise-idle GPSIMD so the DVE/ACT stats pipeline for the
            # next tile isn't blocked; the last tile splits evenly instead
            # because its completion gates the projections
            NSPLIT = 2048 if ct == CT - 1 else 1024
            nc.vector.tensor_scalar(
                out=hslot[:, 0:NSPLIT],
                in0=ta[:, 0:NSPLIT],
                scalar1=scl,
                scalar2=nbs,
                op0=mybir.AluOpType.mult,
                op1=mybir.AluOpType.subtract,
            )
            if NSPLIT < HN:
                nc.gpsimd.tensor_scalar(
                    out=hslot[:, NSPLIT:HN],
                    in0=ta[:, NSPLIT:],
                    scalar1=scl,
                    scalar2=nbs,
                    op0=mybir.AluOpType.mult,
                    op1=mybir.AluOpType.subtract,
                )
            nc.gpsimd.tensor_scalar(
                out=hslot[:, HN:],
                in0=tb[:],
                scalar1=scl,
                scalar2=nbs,
                op0=mybir.AluOpType.mult,
                op1=mybir.AluOpType.subtract,
            )

        # ---- projections (fp8 DoubleRow, contraction 256 per matmul) ------
        for co in range(CT):
            for qc in range(QC):
                ps = ps_s.tile([P, 512], F32, tag="s")
                for j in range(CP):
                    nc.tensor.matmul(
                        ps,
                        qw8[j][:, :, co * P : (co + 1) * P],
                        h8[j][:, :, qc * 512 : (qc + 1) * 512],
                        start=(j == 0),
                        stop=(j == CP - 1),
                        perf_mode=DR,
                    )
                nc.vector.tensor_scalar(
                    out=QT8[co // 2][:, co % 2, qc * 512 : (qc + 1) * 512],
                    in0=ps,
                    scalar1=1.0 / WSCALE,
                    scalar2=vecs_sb[:, 2 * CT + co : 2 * CT + co + 1],
                    op0=mybir.AluOpType.mult,
                    op1=mybir.AluOpType.add,
                )
        # T = Wk^T Q (so S = K^T Q = h^T T): query-sized instead of key-sized,
        # and per-core local.  The k bias shifts every logit of a softmax row
        # equally, so it cancels and is dropped.
        for cb in range(CT):
            for qc in range(QC):
                ps = ps_s.tile([P, 512], F32, tag="s")
                for j in range(CP):
                    nc.tensor.matmul(
                        ps,
                        kw8[j][:, :, cb * P : (cb + 1) * P],
                        QT8[j][:, :, qc * 512 : (qc + 1) * 512],
                        start=(j == 0),
                        stop=(j == CP - 1),
                        perf_mode=DR,
                    )
                nc.vector.tensor_scalar(
                    out=T8[cb // 2][:, cb % 2, qc * 512 : (qc + 1) * 512],
                    in0=ps,
                    scalar1=1.0 / WSCALE,
                    scalar2=None,
                    op0=mybir.AluOpType.mult,
                )
        for nb in range(NKT):
            ps = ps_o.tile([P, 512], F32, tag="o")
            for j in range(CP):
                nc.tensor.matmul(
                    ps,
                    h8[j][:, :, nb * P : (nb + 1) * P],
                    vw8[j][:],
                    start=(j == 0),
                    stop=(j == CP - 1),
                    perf_mode=DR,
                )
            nc.vector.tensor_scalar(
                out=V8[nb // 2][:, nb % 2, :],
                in0=ps,
                scalar1=1.0 / WSCALE,
                scalar2=None,
                op0=mybir.AluOpType.mult,
            )

        xh_ctx.close()

        # ow (fp8, x16) loads after x is freed (SBUF headroom during GN)
        wo_pool = ctx.enter_context(tc.tile_pool(name="wo", bufs=1))
        ow8_t = wo_pool.tile([P, CT, C], FP8, tag="w8_ow", name="ow8")
        nc.sync.dma_start(out=ow8_t[:], in_=owT_d[:])

        # ---- attention ----------------------------------------------------
        attn_ctx = contextlib.ExitStack()
        ppool = attn_ctx.enter_context(tc.tile_pool(name="pT", bufs=20))
        opool = attn_ctx.enter_context(tc.tile_pool(name="oT", bufs=8))
        outp = attn_ctx.enter_context(tc.tile_pool(name="outs", bufs=4))
        invp = attn_ctx.enter_context(tc.tile_pool(name="inv", bufs=2))
        rpool = attn_ctx.enter_context(tc.tile_pool(name="resid", bufs=2))

        def prefetch_resids(qc):
            # issued mid-S-loop so the 1 MB of residual sits in SBUF well
            # before the epilogue's adds need it (host-packed: the whole
            # chunk is one 128-descriptor DMA)
            rt = rpool.tile([P, CT, 512], F32, tag="resid", name=f"rs{qc}")
            nc.sync.dma_start(out=rt[:], in_=xr_d[:, qc * CT : (qc + 1) * CT, :])
            return rt

        def make_epilogue(qc, po, invbc, resids, last=False):
            qs = slice(qc * 512, (qc + 1) * 512)

            def epilogue():
                # OSC/denominator is per-query, so it commutes past the
                # channel contraction of the output projection: fold it into
                # the PSUM->fp8 cast of the raw A.V accumulators (invbc was
                # computed on ACT right when the denominator stopped, so
                # these muls are never blocked) and divide the combined
                # WSCALE*OSC out in the fused residual add after the
                # projection.
                oT8 = [
                    opool.tile([P, 2, 512], FP8, tag="oT", name=f"oT{qc}_{u}")
                    for u in range(2)
                ]
                for cb in range(CT):
                    nc.vector.tensor_mul(
                        out=oT8[cb // 2][:, cb % 2, :], in0=po[cb], in1=invbc[:]
                    )

                # the last chunk's out-proj can use the S pool's PSUM banks
                # (S is done) — two banks instead of one unserializes the
                # four projection groups
                pso_pool = ps_s if last else ps_out
                for cj in range(CT):
                    pso = pso_pool.tile([P, 512], F32, tag="s" if last else "out",
                                        name=f"pso{qc}_{cj}")
                    for u in range(2):
                        nc.tensor.matmul(
                            pso,
                            ow8_t[:, 2 * u : 2 * u + 2, cj * P : (cj + 1) * P],
                            oT8[u][:],
                            start=(u == 0),
                            stop=(u == 1),
                            perf_mode=DR,
                        )
                    ot = outp.tile([P, 512], F32, tag="out_sb", name=f"ot{qc}_{cj}")
                    nc.vector.scalar_tensor_tensor(
                        out=ot[:], in0=pso, scalar=1.0 / (WSCALE * OSC),
                        in1=resids[:, cj, :],
                        op0=mybir.AluOpType.mult, op1=mybir.AluOpType.add,
                    )
                    if last:
                        # nothing left on ACT: split the tail store across
                        # both rings
                        h1 = slice(qc * 512, qc * 512 + 256)
                        h2 = slice(qc * 512 + 256, (qc + 1) * 512)
                        nc.sync.dma_start(
                            out=out_d[cj * P : (cj + 1) * P, h1], in_=ot[:, 0:256]
                        )
                        nc.scalar.dma_start(
                            out=out_d[cj * P : (cj + 1) * P, h2], in_=ot[:, 256:512]
                        )
                    else:
                        nc.sync.dma_start(
                            out=out_d[cj * P : (cj + 1) * P, qs], in_=ot[:]
                        )

            return epilogue

        pending_epilogue = None
        for qc in range(QC):
            qs = slice(qc * 512, (qc + 1) * 512)
            po = [
                ps_o.tile([P, 512], F32, tag="o", name=f"po{qc}_{i}")
                for i in range(CT)
            ]
            psum = ps_stat.tile([P, 512], F32, tag="stat", name=f"psum{qc}")

            def emit_av(pt, m, po=po, psum=psum):
                # denominator matmul after the po matmuls (po banks free
                # first at a chunk boundary) — except the last pair, where
                # it goes first so the reciprocal gets a head start
                if m == NKP - 1:
                    nc.tensor.matmul(
                        psum, ones8, pt[:], start=False, stop=True,
                        perf_mode=DR,
                    )
                for cb in range(CT):
                    nc.tensor.matmul(
                        po[cb],
                        V8[m][:, :, cb * P : (cb + 1) * P],
                        pt[:],
                        start=(m == 0),
                        stop=(m == NKP - 1),
                        perf_mode=DR,
                    )
                if m != NKP - 1:
                    nc.tensor.matmul(
                        psum, ones8, pt[:], start=(m == 0), stop=False,
                        perf_mode=DR,
                    )

            prev = None
            pt = None
            for t in range(NKT):
                ps = ps_s.tile([P, 512], F32, tag="s", name=f"ps{qc}_{t}")
                for j in range(CP):
                    nc.tensor.matmul(
                        ps,
                        h8[j][:, :, t * P : (t + 1) * P],
                        T8[j][:, :, qs],
                        start=(j == 0),
                        stop=(j == CP - 1),
                        perf_mode=DR,
                    )
                if t % 2 == 0:
                    pt = ppool.tile([P, 2, 512], FP8, tag="p", name=f"pt{qc}_{t}")
                nc.scalar.activation(
                    out=pt[:, t % 2, :], in_=ps, func=AF.Exp,
                    bias=esh_sb[:, 0:1], scale=SCALE,
                )
                if t == 2 and pending_epilogue is not None:
                    # run the previous chunk's normalize/out-proj now, so its
                    # reciprocal -> broadcast chain hides under this chunk's
                    # S matmuls
                    pending_epilogue()
                    pending_epilogue = None
                if t == 16:
                    cur_resids = prefetch_resids(qc)
                if t % 2 == 1:
                    if prev is not None:
                        emit_av(*prev)
                    prev = (pt, t // 2)
            emit_av(*prev)
            # OSC/psum = exp(ln(OSC) - ln(psum)) on ACT, emitted now: psum
            # stopped in the last pair's first matmul, so invbc is ready
            # well before the deferred epilogue's casts consume it
            lntmp = invp.tile([P, 512], F32, tag="lntmp", name=f"ln{qc}")
            nc.scalar.activation(out=lntmp[:], in_=psum, func=AF.Ln)
            invbc = invp.tile([P, 512], F32, tag="invbc", name=f"invbc{qc}")
            nc.scalar.activation(
                out=invbc[:], in_=lntmp[:], func=AF.Exp, scale=-1.0,
                bias=lnc_sb[:, 0:1],
            )
            pending_epilogue = make_epilogue(
                qc, po, invbc, cur_resids, last=(qc == QC - 1)
            )
        pending_epilogue()
        attn_ctx.close()

    _split_multi_waits(nc)
    return nc


def _split_multi_waits(nc: bass.Bass):
    """This walrus build encodes at most one sync-wait per instruction; hoist
    extra waits onto NoOps inserted just before the instruction (same engine,
    so per-engine program order enforces them)."""
    k = 0
    for fn in nc.m.functions:
        for bb in fn.blocks:
            new_insts = []
            for inst in bb.instructions:
                si = inst.sync_info
                if si is not None and len(si.on_wait) > 1:
                    waits = list(si.on_wait)
                    for w in waits[:-1]:
                        k += 1
                        new_insts.append(
                            mybir.InstNoOp(
                                name=f"{inst.name}_sw{k}",
                                engine=inst.engine,
                                sync_info=mybir.SyncInfo(on_wait=[w], on_update=[]),
                                bass_nofuse=True,
                            )
                        )
                    inst.sync_info = mybir.SyncInfo(
                        on_wait=[waits[-1]], on_update=list(si.on_update)
                    )
                new_insts.append(inst)
            bb.instructions = new_insts


_NC = None


def _get_nc():
    global _NC
    if _NC is None:
        _NC = _build_nc()
    return _NC


def kernel(x, gn_w, gn_b, qw, qb, kw, kb, vw, vb, ow, ob):
    x = np.asarray(x, dtype=np.float32)
    gn_w = np.asarray(gn_w, dtype=np.float32)
    gn_b = np.asarray(gn_b, dtype=np.float32)
    qb = np.asarray(qb, dtype=np.float32)
    kb = np.asarray(kb, dtype=np.float32)
    ovb = (np.asarray(ow, np.float32) @ np.asarray(vb, np.float32)
           + np.asarray(ob, np.float32)).astype(np.float32)

    ind_np = np.zeros((P, GROUPS_PER_TILE), dtype=np.float32)
    for g in range(GROUPS_PER_TILE):
        ind_np[g * GSIZE : (g + 1) * GSIZE, g] = 1.0
    indT_np = np.zeros((P, P), dtype=np.float32)
    indT_np[:GROUPS_PER_TILE] = ind_np.T

    def pack_rows(m):
        # [512, C] -> [128, 4, C]: [p, t, c] = m[t*128 + p, c]
        return np.ascontiguousarray(m.reshape(CT, P, -1).transpose(1, 0, 2))

    # qw/vw ship transposed (contraction over input channels); kw ships
    # as-is (the T = Wk^T Q matmul contracts over kw's output channels)
    w8s = {
        name: pack_rows(
            (np.asarray(w, np.float32).T * WSCALE).astype(ml_dtypes.float8_e4m3)
        )
        for name, w in (("qw8", qw), ("vw8", vw))
    }
    w8s["kw8"] = pack_rows(
        (np.asarray(kw, np.float32) * WSCALE).astype(ml_dtypes.float8_e4m3)
    )
    owT_np = pack_rows(
        (np.asarray(ow, np.float32).T * WSCALE).astype(ml_dtypes.float8_e4m3)
    )
    vecs_np = np.zeros((P, 12), dtype=np.float32)
    vecs_np[:, 0:CT] = gn_w.reshape(CT, P).T
    vecs_np[:, CT : 2 * CT] = gn_b.reshape(CT, P).T
    vecs_np[:, 2 * CT : 3 * CT] = qb.reshape(CT, P).T

    nc = _get_nc()
    in_maps = []
    for core in range(8):
        b, half = core // 2, core % 2
        xb = np.ascontiguousarray(x[b].reshape(C, N))
        if half == 1:
            xb = np.ascontiguousarray(
                np.concatenate([xb[:, NQ:], xb[:, :NQ]], axis=1)
            )
        xr = xb[:, :NQ] + ovb[:, None]
        # [P, 16, 512]: [p, qc*4 + cj, col] = xr[cj*128 + p, qc*512 + col]
        xrp = np.ascontiguousarray(
            xr.reshape(CT, P, QC, 512).transpose(1, 2, 0, 3).reshape(P, 16, 512)
        )
        in_maps.append(
            {
                "x": xb.astype(ml_dtypes.float8_e4m3),
                "xrp": xrp,
                "vecs": vecs_np,
                "ind": ind_np,
                "indT": indT_np,
                "owT": owT_np,
                **w8s,
            }
        )

    global _last_in_maps
    _last_in_maps = in_maps
    res = run_bass_kernel_spmd(nc, in_maps, list(range(8)))

    out = np.empty((B, C, N), dtype=np.float32)
    for core in range(8):
        b, half = core // 2, core % 2
        sl = slice(0, NQ) if half == 0 else slice(NQ, N)
        out[b][:, sl] = res.results[core]["out"]
    return out.reshape(B, C, H, W)
